# revision 21
# baseline (speedup 1.0000x reference)
import sys
if "/opt/trn_rl_repo" not in sys.path:
    sys.path.insert(0, "/opt/trn_rl_repo")

import numpy as np
import concourse.bacc as bacc
import concourse.tile as tile
from concourse import mybir
from concourse.bass_utils import run_bass_kernel_spmd

B, S, D = 4, 2048, 1024
NCORES = 8
F32 = mybir.dt.float32
F32R = mybir.dt.float32r
_cache = {}


def _build(reps=1):
    if reps in _cache:
        return _cache[reps]
    nc = bacc.Bacc()
    xt = nc.dram_tensor("xt", [D, B * S], F32R, kind="ExternalInput")
    wq = nc.dram_tensor("wq", [128, D], F32R, kind="ExternalInput")
    wk = nc.dram_tensor("wk", [128, D], F32R, kind="ExternalInput")
    wv = nc.dram_tensor("wv", [128, D], F32R, kind="ExternalInput")
    wo = nc.dram_tensor("wo", [128, D], F32R, kind="ExternalInput")
    bq = nc.dram_tensor("bq", [128, 1], F32, kind="ExternalInput")
    bk = nc.dram_tensor("bk", [128, 1], F32, kind="ExternalInput")
    bv = nc.dram_tensor("bv", [128, 1], F32, kind="ExternalInput")
    idm = nc.dram_tensor("idm", [128, 128], F32R, kind="ExternalInput")
    on32 = nc.dram_tensor("on32", [128, 32], F32R, kind="ExternalInput")
    po = nc.dram_tensor("po", [B * D, S], F32, kind="ExternalOutput")

    ACT = mybir.ActivationFunctionType

    with tile.TileContext(nc) as tc:
        with tc.tile_pool(name="sb", bufs=1) as sb, \
             tc.tile_pool(name="ps", bufs=2, space="PSUM") as ps:
            wq_sb = sb.tile([128, D], F32R)
            wk_sb = sb.tile([128, D], F32R)
            wv_sb = sb.tile([128, D], F32R)
            wo_sb = sb.tile([128, D], F32R)
            bq_sb = sb.tile([128, 1], F32)
            bk_sb = sb.tile([128, 1], F32)
            bv_sb = sb.tile([128, 1], F32)
            nc.sync.dma_start(out=wq_sb, in_=wq[:, :])
            nc.sync.dma_start(out=wk_sb, in_=wk[:, :])
            nc.sync.dma_start(out=wv_sb, in_=wv[:, :])
            nc.sync.dma_start(out=wo_sb, in_=wo[:, :])
            nc.sync.dma_start(out=bq_sb, in_=bq[:, :])
            nc.sync.dma_start(out=bk_sb, in_=bk[:, :])
            nc.sync.dma_start(out=bv_sb, in_=bv[:, :])

            ident = sb.tile([128, 128], F32R)
            nc.sync.dma_start(out=ident, in_=idm[:, :])
            ones_sb = sb.tile([1, 64], F32)
            nc.vector.memset(ones_sb[:, :], 1.0)
            # vp: 16 sk-tiles x (64 V_h0 | ones | 64 V_h1 | ones) = 130 cols each
            vp = sb.tile([128, 16 * 130], F32R)
            nc.sync.dma_start(out=vp[:, 64:16 * 130:65], in_=on32[:, :])

            qt = sb.tile([128, S], F32R)
            kt = sb.tile([128, S], F32R)
            vt = sb.tile([128, S], F32R)
            ctxT = sb.tile([128, S], F32R)

            for b in list(range(B)) * reps:
                xsl = []
                for k in range(8):
                    xs = sb.tile([128, S], F32R, tag="xs", bufs=10)
                    nc.sync.dma_start(
                        out=xs, in_=xt[k * 128:(k + 1) * 128, b * S:(b + 1) * S])
                    xsl.append(xs)

                # QKV projections: dst[:, half] = w.T @ x (K=1024 accumulated)
                for wt, bt, dst in ((wq_sb, bq_sb, qt), (wk_sb, bk_sb, kt),
                                    (wv_sb, bv_sb, vt)):
                    for half in range(2):
                        pq = ps.tile([128, 1024], F32, tag="pa", bufs=2)
                        for n2 in range(2):
                            c0 = half * 1024 + n2 * 512
                            for k in range(8):
                                nc.tensor.matmul(
                                    pq[:, n2 * 512:(n2 + 1) * 512],
                                    wt[:, k * 128:(k + 1) * 128],
                                    xsl[k][:, c0:c0 + 512],
                                    start=(k == 0), stop=(k == 7))
                        nc.scalar.activation(
                            out=dst[:, half * 1024:(half + 1) * 1024], in_=pq,
                            func=ACT.Identity, bias=bt[:, 0:1], scale=1.0)

                # V' build: transpose VT tiles into vp (ones cols persist)
                for t in range(16):
                    tp = ps.tile([128, 1024], F32R, tag="pa", bufs=2)
                    nc.tensor.transpose(
                        tp[:, 0:128], vt[:, t * 128:(t + 1) * 128], ident[:, :])
                    nc.vector.tensor_copy(
                        out=vp[:, t * 130:t * 130 + 64], in_=tp[:, 0:64])
                    nc.vector.tensor_copy(
                        out=vp[:, t * 130 + 65:t * 130 + 129], in_=tp[:, 64:128])

                # attention: flattened (h,j,t) pipeline, PV lags scores by
                # one stage so PE fills ACT's exp latency with work
                ets = {}
                cxps = {}
                for s in range(65):
                    if s < 64:
                        p, t = s // 16, s % 16
                        h, j = p // 2, p % 2
                        scp = ps.tile([128, 1024], F32, tag="pa", bufs=2)
                        for n2 in range(2):
                            q0 = j * 1024 + n2 * 512
                            nc.tensor.matmul(
                                scp[:, n2 * 512:(n2 + 1) * 512],
                                kt[h * 64:(h + 1) * 64, t * 128:(t + 1) * 128],
                                qt[h * 64:(h + 1) * 64, q0:q0 + 512],
                                start=True, stop=True)
                        et = sb.tile([128, 1024], F32R, tag="et", bufs=3)
                        nc.scalar.activation(
                            out=et, in_=scp, func=ACT.Exp, scale=0.125)
                        ets[s] = et
                    if s >= 1:
                        p1, t1 = (s - 1) // 16, (s - 1) % 16
                        h1 = p1 // 2
                        if t1 == 0:
                            cxp_new = ps.tile([128, 1024], F32, tag="cx",
                                              bufs=2)
                            cxps[p1] = cxp_new
                        et1 = ets.pop(s - 1)
                        for n2 in range(2):
                            nc.tensor.matmul(
                                cxps[p1][0:65, n2 * 512:(n2 + 1) * 512],
                                vp[:, t1 * 130 + h1 * 65:t1 * 130 + (h1 + 1) * 65],
                                et1[:, n2 * 512:(n2 + 1) * 512],
                                start=(t1 == 0), stop=(t1 == 15))
                        if t1 == 15:
                            j1 = p1 % 2
                            cxp = cxps.pop(p1)
                            rc = sb.tile([1, 1024], F32, tag="rc", bufs=2)
                            nc.vector.reciprocal(rc[0:1, :], cxp[64:65, :])
                            bcp = ps.tile([128, 1024], F32, tag="cx", bufs=2)
                            for n2 in range(2):
                                nc.tensor.matmul(
                                    bcp[0:64, n2 * 512:(n2 + 1) * 512],
                                    ones_sb[0:1, 0:64],
                                    rc[0:1, n2 * 512:(n2 + 1) * 512],
                                    start=True, stop=True)
                            bcs = sb.tile([64, 1024], F32, tag="bcs", bufs=2)
                            nc.vector.tensor_copy(out=bcs, in_=bcp[0:64, :])
                            nc.vector.tensor_tensor(
                                ctxT[h1 * 64:(h1 + 1) * 64,
                                     j1 * 1024:(j1 + 1) * 1024],
                                cxp[0:64, :], bcs[:, :], mybir.AluOpType.mult)

                # out projection: po[b] += woT @ ctxT (partial, host-reduced)
                for m in range(8):
                    for half in range(2):
                        pso = ps.tile([128, 1024], F32, tag="pa", bufs=2)
                        for n2 in range(2):
                            c0 = half * 1024 + n2 * 512
                            nc.tensor.matmul(
                                pso[:, n2 * 512:(n2 + 1) * 512],
                                wo_sb[:, m * 128:(m + 1) * 128],
                                ctxT[:, c0:c0 + 512], start=True, stop=True)
                        ob = sb.tile([128, 1024], F32, tag="ob", bufs=4)
                        nc.vector.tensor_copy(out=ob, in_=pso)
                        nc.sync.dma_start(
                            out=po[b * D + m * 128:b * D + (m + 1) * 128,
                                   half * 1024:(half + 1) * 1024],
                            in_=ob)
    nc.finalize()
    _cache[reps] = nc
    return nc


def _warr(W):
    # W [128 outdims, 1024 indims] -> SBUF lhsT layout [128 p, 8k x 128 m]
    return np.ascontiguousarray(
        W.reshape(128, 8, 128).transpose(2, 1, 0).reshape(128, 1024))


def _in_maps(x, qkv_w, qkv_b, out_w):
    xT = np.ascontiguousarray(
        x.reshape(B * S, D).T.astype(np.float32))
    in_maps = []
    for c in range(NCORES):
        base = c * 128
        V = out_w[:, base:base + 128]
        in_maps.append({
            "xt": xT,
            "wq": _warr(qkv_w[base:base + 128, :].astype(np.float32)),
            "wk": _warr(qkv_w[D + base:D + base + 128, :].astype(np.float32)),
            "wv": _warr(qkv_w[2 * D + base:2 * D + base + 128, :].astype(np.float32)),
            "wo": np.ascontiguousarray(
                V.reshape(8, 128, 128).transpose(2, 0, 1).reshape(128, 1024)
            ).astype(np.float32),
            "idm": np.eye(128, dtype=np.float32),
            "on32": np.ones((128, 32), dtype=np.float32),
            "bq": qkv_b[base:base + 128].reshape(128, 1).astype(np.float32),
            "bk": qkv_b[D + base:D + base + 128].reshape(128, 1).astype(np.float32),
            "bv": qkv_b[2 * D + base:2 * D + base + 128].reshape(128, 1).astype(np.float32),
        })
    return in_maps


def kernel(x, qkv_w, qkv_b, out_w, out_b):
    nc = _build()
    in_maps = _in_maps(x, qkv_w, qkv_b, out_w)
    res = run_bass_kernel_spmd(nc, in_maps, core_ids=list(range(NCORES)),
                               trace=False)
    kernel.last_exec_ns = res.exec_time_ns
    acc = np.zeros((B, D, S), dtype=np.float64)
    for c in range(NCORES):
        acc += res.results[c]["po"].reshape(B, D, S)
    out = acc.transpose(0, 2, 1) + out_b.astype(np.float64)
    return out.astype(np.float32)


# revision 27
# speedup vs baseline: 1.0173x; 1.0173x over previous
import sys
if "/opt/trn_rl_repo" not in sys.path:
    sys.path.insert(0, "/opt/trn_rl_repo")

import numpy as np
import ml_dtypes
import concourse.bacc as bacc
import concourse.tile as tile
from concourse import mybir
from concourse.bass_utils import run_bass_kernel_spmd

B, S, D = 4, 2048, 1024
NCORES = 8
F32 = mybir.dt.float32
F32R = mybir.dt.float32r
BF16 = mybir.dt.bfloat16
_cache = {}


def _build(reps=1):
    if reps in _cache:
        return _cache[reps]
    nc = bacc.Bacc()
    xt = nc.dram_tensor("xt", [D, B * S], F32R, kind="ExternalInput")
    wq = nc.dram_tensor("wq", [128, D], F32R, kind="ExternalInput")
    wk = nc.dram_tensor("wk", [128, D], F32R, kind="ExternalInput")
    wv = nc.dram_tensor("wv", [128, D], F32R, kind="ExternalInput")
    wo = nc.dram_tensor("wo", [128, D], F32R, kind="ExternalInput")
    bq = nc.dram_tensor("bq", [128, 1], F32, kind="ExternalInput")
    bk = nc.dram_tensor("bk", [128, 1], F32, kind="ExternalInput")
    bv = nc.dram_tensor("bv", [128, 1], F32, kind="ExternalInput")
    idm = nc.dram_tensor("idm", [128, 128], F32R, kind="ExternalInput")
    on32 = nc.dram_tensor("on32", [128, 32], BF16, kind="ExternalInput")
    po = nc.dram_tensor("po", [B * D, S], F32, kind="ExternalOutput")

    ACT = mybir.ActivationFunctionType

    with tile.TileContext(nc) as tc:
        with tc.tile_pool(name="sb", bufs=1) as sb, \
             tc.tile_pool(name="ps", bufs=2, space="PSUM") as ps:
            wq_sb = sb.tile([128, D], F32R)
            wk_sb = sb.tile([128, D], F32R)
            wv_sb = sb.tile([128, D], F32R)
            wo_sb = sb.tile([128, D], F32R)
            bq_sb = sb.tile([128, 1], F32)
            bk_sb = sb.tile([128, 1], F32)
            bv_sb = sb.tile([128, 1], F32)
            nc.sync.dma_start(out=wq_sb, in_=wq[:, :])
            nc.sync.dma_start(out=wk_sb, in_=wk[:, :])
            nc.sync.dma_start(out=wv_sb, in_=wv[:, :])
            nc.sync.dma_start(out=wo_sb, in_=wo[:, :])
            nc.sync.dma_start(out=bq_sb, in_=bq[:, :])
            nc.sync.dma_start(out=bk_sb, in_=bk[:, :])
            nc.sync.dma_start(out=bv_sb, in_=bv[:, :])

            ident = sb.tile([128, 128], F32R)
            nc.sync.dma_start(out=ident, in_=idm[:, :])
            ones_sb = sb.tile([1, 64], F32)
            nc.vector.memset(ones_sb[:, :], 1.0)
            # vp: 16 sk-tiles x (64 V_h0 | ones | 64 V_h1 | ones) = 130 cols each
            vp = sb.tile([128, 16 * 130], BF16)
            nc.sync.dma_start(out=vp[:, 64:16 * 130:65], in_=on32[:, :])

            qt = sb.tile([128, S], F32R)
            kt = sb.tile([128, S], F32R)
            vt = sb.tile([128, S], F32R)
            ctxT = sb.tile([128, S], F32R)

            for b in list(range(B)) * reps:
                xsl = []
                for k in range(8):
                    xs = sb.tile([128, S], F32R, tag="xs", bufs=10)
                    nc.sync.dma_start(
                        out=xs, in_=xt[k * 128:(k + 1) * 128, b * S:(b + 1) * S])
                    xsl.append(xs)

                # QKV projections: dst[:, half] = w.T @ x (K=1024 accumulated)
                for wt, bt, dst in ((wq_sb, bq_sb, qt), (wk_sb, bk_sb, kt),
                                    (wv_sb, bv_sb, vt)):
                    for half in range(2):
                        pq = ps.tile([128, 1024], F32, tag="pa", bufs=2)
                        for n2 in range(2):
                            c0 = half * 1024 + n2 * 512
                            for k in range(8):
                                nc.tensor.matmul(
                                    pq[:, n2 * 512:(n2 + 1) * 512],
                                    wt[:, k * 128:(k + 1) * 128],
                                    xsl[k][:, c0:c0 + 512],
                                    start=(k == 0), stop=(k == 7))
                        nc.scalar.activation(
                            out=dst[:, half * 1024:(half + 1) * 1024], in_=pq,
                            func=ACT.Identity, bias=bt[:, 0:1], scale=1.0)

                # V' build: transpose VT tiles into vp (ones cols persist)
                for t in range(16):
                    tp = ps.tile([128, 1024], F32R, tag="pa", bufs=2)
                    nc.tensor.transpose(
                        tp[:, 0:128], vt[:, t * 128:(t + 1) * 128], ident[:, :])
                    nc.vector.tensor_copy(
                        out=vp[:, t * 130:t * 130 + 64], in_=tp[:, 0:64])
                    nc.vector.tensor_copy(
                        out=vp[:, t * 130 + 65:t * 130 + 129], in_=tp[:, 64:128])

                # attention: flattened (h,j,t) pipeline, PV lags scores by
                # one stage so PE fills ACT's exp latency with work
                ets = {}
                cxps = {}
                for s in range(65):
                    if s < 64:
                        p, t = s // 16, s % 16
                        h, j = p // 2, p % 2
                        scp = ps.tile([128, 1024], F32, tag="pa", bufs=2)
                        for n2 in range(2):
                            q0 = j * 1024 + n2 * 512
                            nc.tensor.matmul(
                                scp[:, n2 * 512:(n2 + 1) * 512],
                                kt[h * 64:(h + 1) * 64, t * 128:(t + 1) * 128],
                                qt[h * 64:(h + 1) * 64, q0:q0 + 512],
                                start=True, stop=True)
                        et = sb.tile([128, 1024], BF16, tag="et", bufs=3)
                        nc.scalar.activation(
                            out=et, in_=scp, func=ACT.Exp, scale=0.125)
                        ets[s] = et
                    if s >= 1:
                        p1, t1 = (s - 1) // 16, (s - 1) % 16
                        h1 = p1 // 2
                        if t1 == 0:
                            cxp_new = ps.tile([128, 1024], F32, tag="cx",
                                              bufs=2)
                            cxps[p1] = cxp_new
                        et1 = ets.pop(s - 1)
                        for n2 in range(2):
                            nc.tensor.matmul(
                                cxps[p1][0:65, n2 * 512:(n2 + 1) * 512],
                                vp[:, t1 * 130 + h1 * 65:t1 * 130 + (h1 + 1) * 65],
                                et1[:, n2 * 512:(n2 + 1) * 512],
                                start=(t1 == 0), stop=(t1 == 15))
                        if t1 == 15:
                            j1 = p1 % 2
                            cxp = cxps.pop(p1)
                            rc = sb.tile([1, 1024], F32, tag="rc", bufs=2)
                            nc.vector.reciprocal(rc[0:1, :], cxp[64:65, :])
                            bcp = ps.tile([128, 1024], F32, tag="cx", bufs=2)
                            for n2 in range(2):
                                nc.tensor.matmul(
                                    bcp[0:64, n2 * 512:(n2 + 1) * 512],
                                    ones_sb[0:1, 0:64],
                                    rc[0:1, n2 * 512:(n2 + 1) * 512],
                                    start=True, stop=True)
                            bcs = sb.tile([64, 1024], F32, tag="bcs", bufs=2)
                            nc.vector.tensor_copy(out=bcs, in_=bcp[0:64, :])
                            nc.vector.tensor_tensor(
                                ctxT[h1 * 64:(h1 + 1) * 64,
                                     j1 * 1024:(j1 + 1) * 1024],
                                cxp[0:64, :], bcs[:, :], mybir.AluOpType.mult)

                # out projection: po[b] += woT @ ctxT (partial, host-reduced)
                for m in range(8):
                    for half in range(2):
                        pso = ps.tile([128, 1024], F32, tag="pa", bufs=2)
                        for n2 in range(2):
                            c0 = half * 1024 + n2 * 512
                            nc.tensor.matmul(
                                pso[:, n2 * 512:(n2 + 1) * 512],
                                wo_sb[:, m * 128:(m + 1) * 128],
                                ctxT[:, c0:c0 + 512], start=True, stop=True)
                        ob = sb.tile([128, 1024], F32, tag="ob", bufs=4)
                        nc.vector.tensor_copy(out=ob, in_=pso)
                        nc.sync.dma_start(
                            out=po[b * D + m * 128:b * D + (m + 1) * 128,
                                   half * 1024:(half + 1) * 1024],
                            in_=ob)
    nc.finalize()
    _cache[reps] = nc
    return nc


def _warr(W):
    # W [128 outdims, 1024 indims] -> SBUF lhsT layout [128 p, 8k x 128 m]
    return np.ascontiguousarray(
        W.reshape(128, 8, 128).transpose(2, 1, 0).reshape(128, 1024))


def _in_maps(x, qkv_w, qkv_b, out_w):
    xT = np.ascontiguousarray(
        x.reshape(B * S, D).T.astype(np.float32))
    in_maps = []
    for c in range(NCORES):
        base = c * 128
        V = out_w[:, base:base + 128]
        in_maps.append({
            "xt": xT,
            "wq": _warr(qkv_w[base:base + 128, :].astype(np.float32)),
            "wk": _warr(qkv_w[D + base:D + base + 128, :].astype(np.float32)),
            "wv": _warr(qkv_w[2 * D + base:2 * D + base + 128, :].astype(np.float32)),
            "wo": np.ascontiguousarray(
                V.reshape(8, 128, 128).transpose(2, 0, 1).reshape(128, 1024)
            ).astype(np.float32),
            "idm": np.eye(128, dtype=np.float32),
            "on32": np.ones((128, 32), dtype=ml_dtypes.bfloat16),
            "bq": qkv_b[base:base + 128].reshape(128, 1).astype(np.float32),
            "bk": qkv_b[D + base:D + base + 128].reshape(128, 1).astype(np.float32),
            "bv": qkv_b[2 * D + base:2 * D + base + 128].reshape(128, 1).astype(np.float32),
        })
    return in_maps


def kernel(x, qkv_w, qkv_b, out_w, out_b):
    nc = _build()
    in_maps = _in_maps(x, qkv_w, qkv_b, out_w)
    res = run_bass_kernel_spmd(nc, in_maps, core_ids=list(range(NCORES)),
                               trace=False)
    kernel.last_exec_ns = res.exec_time_ns
    acc = np.zeros((B, D, S), dtype=np.float64)
    for c in range(NCORES):
        acc += res.results[c]["po"].reshape(B, D, S)
    out = acc.transpose(0, 2, 1) + out_b.astype(np.float64)
    return out.astype(np.float32)


# revision 28
# speedup vs baseline: 18.6494x; 18.3327x over previous
import sys
if "/opt/trn_rl_repo" not in sys.path:
    sys.path.insert(0, "/opt/trn_rl_repo")

import numpy as np
import concourse.bacc as bacc
import concourse.tile as tile
from concourse import mybir
from concourse.bass_utils import run_bass_kernel_spmd

B, S, D = 4, 2048, 1024
NCORES = 8
F32 = mybir.dt.float32
F32R = mybir.dt.float32r
_cache = {}


def _build(reps=1):
    if reps in _cache:
        return _cache[reps]
    nc = bacc.Bacc()
    xt = nc.dram_tensor("xt", [D, B * S], F32R, kind="ExternalInput")
    wq = nc.dram_tensor("wq", [128, D], F32R, kind="ExternalInput")
    wk = nc.dram_tensor("wk", [128, D], F32R, kind="ExternalInput")
    wv = nc.dram_tensor("wv", [128, D], F32R, kind="ExternalInput")
    wo = nc.dram_tensor("wo", [128, D], F32R, kind="ExternalInput")
    bq = nc.dram_tensor("bq", [128, 1], F32, kind="ExternalInput")
    bk = nc.dram_tensor("bk", [128, 1], F32, kind="ExternalInput")
    bv = nc.dram_tensor("bv", [128, 1], F32, kind="ExternalInput")
    idm = nc.dram_tensor("idm", [128, 128], F32R, kind="ExternalInput")
    on32 = nc.dram_tensor("on32", [128, 32], F32R, kind="ExternalInput")
    po = nc.dram_tensor("po", [B * D, S], F32, kind="ExternalOutput")

    ACT = mybir.ActivationFunctionType

    with tile.TileContext(nc) as tc:
        with tc.tile_pool(name="sb", bufs=1) as sb, \
             tc.tile_pool(name="ps", bufs=2, space="PSUM") as ps:
            wq_sb = sb.tile([128, D], F32R)
            wk_sb = sb.tile([128, D], F32R)
            wv_sb = sb.tile([128, D], F32R)
            wo_sb = sb.tile([128, D], F32R)
            bq_sb = sb.tile([128, 1], F32)
            bk_sb = sb.tile([128, 1], F32)
            bv_sb = sb.tile([128, 1], F32)
            nc.sync.dma_start(out=wq_sb, in_=wq[:, :])
            nc.sync.dma_start(out=wk_sb, in_=wk[:, :])
            nc.sync.dma_start(out=wv_sb, in_=wv[:, :])
            nc.sync.dma_start(out=wo_sb, in_=wo[:, :])
            nc.sync.dma_start(out=bq_sb, in_=bq[:, :])
            nc.sync.dma_start(out=bk_sb, in_=bk[:, :])
            nc.sync.dma_start(out=bv_sb, in_=bv[:, :])

            ident = sb.tile([128, 128], F32R)
            nc.sync.dma_start(out=ident, in_=idm[:, :])
            ones_sb = sb.tile([1, 64], F32)
            nc.vector.memset(ones_sb[:, :], 1.0)
            # vp: 16 sk-tiles x (64 V_h0 | ones | 64 V_h1 | ones) = 130 cols each
            vp = sb.tile([128, 16 * 130], F32R)
            nc.sync.dma_start(out=vp[:, 64:16 * 130:65], in_=on32[:, :])

            qt = sb.tile([128, S], F32R)
            kt = sb.tile([128, S], F32R)
            vt = sb.tile([128, S], F32R)
            ctxT = sb.tile([128, S], F32R)

            for b in list(range(B)) * reps:
                xsl = []
                for k in range(8):
                    xs = sb.tile([128, S], F32R, tag="xs", bufs=10)
                    nc.sync.dma_start(
                        out=xs, in_=xt[k * 128:(k + 1) * 128, b * S:(b + 1) * S])
                    xsl.append(xs)

                # QKV projections: dst[:, half] = w.T @ x (K=1024 accumulated)
                for wt, bt, dst in ((wq_sb, bq_sb, qt), (wk_sb, bk_sb, kt),
                                    (wv_sb, bv_sb, vt)):
                    for half in range(2):
                        pq = ps.tile([128, 1024], F32, tag="pa", bufs=2)
                        for n2 in range(2):
                            c0 = half * 1024 + n2 * 512
                            for k in range(8):
                                nc.tensor.matmul(
                                    pq[:, n2 * 512:(n2 + 1) * 512],
                                    wt[:, k * 128:(k + 1) * 128],
                                    xsl[k][:, c0:c0 + 512],
                                    start=(k == 0), stop=(k == 7))
                        nc.scalar.activation(
                            out=dst[:, half * 1024:(half + 1) * 1024], in_=pq,
                            func=ACT.Identity, bias=bt[:, 0:1], scale=1.0)

                # V' build: transpose VT tiles into vp (ones cols persist)
                for t in range(16):
                    tp = ps.tile([128, 1024], F32R, tag="pa", bufs=2)
                    nc.tensor.transpose(
                        tp[:, 0:128], vt[:, t * 128:(t + 1) * 128], ident[:, :])
                    nc.vector.tensor_copy(
                        out=vp[:, t * 130:t * 130 + 64], in_=tp[:, 0:64])
                    nc.vector.tensor_copy(
                        out=vp[:, t * 130 + 65:t * 130 + 129], in_=tp[:, 64:128])

                # attention: flattened (h,j,t) pipeline, PV lags scores by
                # one stage so PE fills ACT's exp latency with work
                ets = {}
                cxps = {}
                for s in range(65):
                    if s < 64:
                        p, t = s // 16, s % 16
                        h, j = p // 2, p % 2
                        scp = ps.tile([128, 1024], F32, tag="pa", bufs=2)
                        for n2 in range(2):
                            q0 = j * 1024 + n2 * 512
                            nc.tensor.matmul(
                                scp[:, n2 * 512:(n2 + 1) * 512],
                                kt[h * 64:(h + 1) * 64, t * 128:(t + 1) * 128],
                                qt[h * 64:(h + 1) * 64, q0:q0 + 512],
                                start=True, stop=True)
                        et = sb.tile([128, 1024], F32R, tag="et", bufs=3)
                        nc.scalar.activation(
                            out=et, in_=scp, func=ACT.Exp, scale=0.125)
                        ets[s] = et
                    if s >= 1:
                        p1, t1 = (s - 1) // 16, (s - 1) % 16
                        h1 = p1 // 2
                        if t1 == 0:
                            cxp_new = ps.tile([128, 1024], F32, tag="cx",
                                              bufs=2)
                            cxps[p1] = cxp_new
                        et1 = ets.pop(s - 1)
                        for n2 in range(2):
                            nc.tensor.matmul(
                                cxps[p1][0:65, n2 * 512:(n2 + 1) * 512],
                                vp[:, t1 * 130 + h1 * 65:t1 * 130 + (h1 + 1) * 65],
                                et1[:, n2 * 512:(n2 + 1) * 512],
                                start=(t1 == 0), stop=(t1 == 15))
                        if t1 == 15:
                            j1 = p1 % 2
                            cxp = cxps.pop(p1)
                            rc = sb.tile([1, 1024], F32, tag="rc", bufs=2)
                            nc.vector.reciprocal(rc[0:1, :], cxp[64:65, :])
                            bcp = ps.tile([128, 1024], F32, tag="cx", bufs=2)
                            for n2 in range(2):
                                nc.tensor.matmul(
                                    bcp[0:64, n2 * 512:(n2 + 1) * 512],
                                    ones_sb[0:1, 0:64],
                                    rc[0:1, n2 * 512:(n2 + 1) * 512],
                                    start=True, stop=True)
                            bcs = sb.tile([64, 1024], F32, tag="bcs", bufs=2)
                            nc.vector.tensor_copy(out=bcs, in_=bcp[0:64, :])
                            nc.vector.tensor_tensor(
                                ctxT[h1 * 64:(h1 + 1) * 64,
                                     j1 * 1024:(j1 + 1) * 1024],
                                cxp[0:64, :], bcs[:, :], mybir.AluOpType.mult)

                # out projection: po[b] += woT @ ctxT (partial, host-reduced)
                for m in range(8):
                    for half in range(2):
                        pso = ps.tile([128, 1024], F32, tag="pa", bufs=2)
                        for n2 in range(2):
                            c0 = half * 1024 + n2 * 512
                            nc.tensor.matmul(
                                pso[:, n2 * 512:(n2 + 1) * 512],
                                wo_sb[:, m * 128:(m + 1) * 128],
                                ctxT[:, c0:c0 + 512], start=True, stop=True)
                        ob = sb.tile([128, 1024], F32, tag="ob", bufs=4)
                        nc.vector.tensor_copy(out=ob, in_=pso)
                        nc.sync.dma_start(
                            out=po[b * D + m * 128:b * D + (m + 1) * 128,
                                   half * 1024:(half + 1) * 1024],
                            in_=ob)
    nc.finalize()
    _cache[reps] = nc
    return nc


def _warr(W):
    # W [128 outdims, 1024 indims] -> SBUF lhsT layout [128 p, 8k x 128 m]
    return np.ascontiguousarray(
        W.reshape(128, 8, 128).transpose(2, 1, 0).reshape(128, 1024))


def _in_maps(x, qkv_w, qkv_b, out_w):
    xT = np.ascontiguousarray(
        x.reshape(B * S, D).T.astype(np.float32))
    in_maps = []
    for c in range(NCORES):
        base = c * 128
        V = out_w[:, base:base + 128]
        in_maps.append({
            "xt": xT,
            "wq": _warr(qkv_w[base:base + 128, :].astype(np.float32)),
            "wk": _warr(qkv_w[D + base:D + base + 128, :].astype(np.float32)),
            "wv": _warr(qkv_w[2 * D + base:2 * D + base + 128, :].astype(np.float32)),
            "wo": np.ascontiguousarray(
                V.reshape(8, 128, 128).transpose(2, 0, 1).reshape(128, 1024)
            ).astype(np.float32),
            "idm": np.eye(128, dtype=np.float32),
            "on32": np.ones((128, 32), dtype=np.float32),
            "bq": qkv_b[base:base + 128].reshape(128, 1).astype(np.float32),
            "bk": qkv_b[D + base:D + base + 128].reshape(128, 1).astype(np.float32),
            "bv": qkv_b[2 * D + base:2 * D + base + 128].reshape(128, 1).astype(np.float32),
        })
    return in_maps


def kernel(x, qkv_w, qkv_b, out_w, out_b):
    nc = _build()
    in_maps = _in_maps(x, qkv_w, qkv_b, out_w)
    res = run_bass_kernel_spmd(nc, in_maps, core_ids=list(range(NCORES)),
                               trace=False)
    kernel.last_exec_ns = res.exec_time_ns
    acc = np.zeros((B, D, S), dtype=np.float64)
    for c in range(NCORES):
        acc += res.results[c]["po"].reshape(B, D, S)
    out = acc.transpose(0, 2, 1) + out_b.astype(np.float64)
    return out.astype(np.float32)
